# revision 4
# baseline (speedup 1.0000x reference)
"""Bilateral filter (cv2 semantics: d=9, sigmaColor=sigmaSpace=75, reflect-101
border, inscribed-circle taps, L1 color distance) on 8 Trainium2 NeuronCores.

Contract: kernel(sample=np.float32[8,1024,1024,3]) -> np.float32[8,1024,1024,3].
Data parallel: one image per core. Device kernel computes, per tap pair
(t,-t), the shared weight field W_t (exp of squared L1 color distance) and a
product field G_c = W_t * (I_c(.+t) - I_c(.)), then accumulates the residual
form  out = center + (sum_t W_t d_t)/(sum_t W_t)  using gather(+G) and
scatter(-shifted G) contributions, so each pair's distance/exp pipeline is
computed once for both taps.

Layout: all tap shifts live in the free dimension. Each of 128 partitions owns
a [64,32] output block (16 row-bands x 8 col-blocks) and loads a [72,40] halo
window; the image is processed in 4 column chunks. Host pre-pads (reflect),
converts to bf16 and extracts halo windows; the device does all filtering.
"""

import os
import sys

for _p in ("/opt/trn_rl_repo", "/root/.axon_site/_ro/trn_rl_repo"):
    if os.path.isdir(_p) and _p not in sys.path:
        sys.path.insert(0, _p)

import numpy as np
import ml_dtypes

import concourse.bacc as bacc
import concourse.mybir as mybir
import concourse.tile as tile
from concourse.bass_utils import run_bass_kernel_spmd

BF16 = ml_dtypes.bfloat16

# Filter constants (must match the reference).
D = 9
R = D // 2  # 4
SIGMA_COLOR = 75.0
SIGMA_SPACE = 75.0
COLOR_COEFF = -0.5 / (SIGMA_COLOR * SIGMA_COLOR)
SPACE_COEFF = -0.5 / (SIGMA_SPACE * SIGMA_SPACE)

B, H, W, C = 8, 1024, 1024, 3
N_CORES = 8

# Device geometry: per chunk, 128 partitions = 16 row-bands x 8 col-blocks,
# each owning a [BR, BC] output block with an [BR+2R, BC+2R] halo window.
BR, BC = 64, 32
EY, EX = BR + 2 * R, BC + 2 * R  # 72, 40
ROW_BANDS = H // BR  # 16
COL_BLOCKS_PER_CHUNK = 128 // ROW_BANDS  # 8
CHUNKS = W // (BC * COL_BLOCKS_PER_CHUNK)  # 4
F_EXT = EY * EX
F_OUT = BR * BC

# Symmetric tap pairs of the inscribed-circle 9x9 stencil: (dy,dx) with
# dy>0, or dy==0 and dx>0. The center tap is implicit (d=0, w=1).
PAIRS = [
    (dy, dx)
    for dy in range(0, R + 1)
    for dx in range(-R, R + 1)
    if dy * dy + dx * dx <= R * R and (dy > 0 or dx > 0)
]
assert len(PAIRS) == 24


def _space_weight(dy, dx):
    return float(np.exp(SPACE_COEFF * (dy * dy + dx * dx)).astype(np.float32))


def _build_nc():
    """Build + compile the per-core Bass program once."""
    nc = bacc.Bacc(None, target_bir_lowering=False)
    # Register const APs for the activation bias values ln(space_weight).
    for _dy, _dx in PAIRS:
        v = float(np.log(_space_weight(_dy, _dx)))
        if (mybir.dt.float32, v) not in nc.const_aps.aps:
            t = nc.alloc_sbuf_tensor(f"const-lnsw-{_dy}-{_dx}", [128, 1], mybir.dt.float32)
            nc.gpsimd.memset(t.ap(), v)
            nc.const_aps.aps[(mybir.dt.float32, v)] = t.ap()
    nc.all_engine_barrier()
    inp = nc.declare_dram_parameter(
        "win", [CHUNKS, C, 128, EY, EX], mybir.dt.bfloat16, isOutput=False
    )
    outp = nc.declare_dram_parameter(
        "out", [CHUNKS, C, 128, BR, BC], mybir.dt.bfloat16, isOutput=True
    )

    bf16 = mybir.dt.bfloat16
    f32 = mybir.dt.float32
    Alu = mybir.AluOpType
    Act = mybir.ActivationFunctionType

    with tile.TileContext(nc) as tc:
        with (
            tc.tile_pool(name="img", bufs=2) as img_pool,
            tc.tile_pool(name="dpool", bufs=2) as d_pool,
            tc.tile_pool(name="wpip", bufs=2) as w_pool,
            tc.tile_pool(name="gpool", bufs=2) as g_pool,
            tc.tile_pool(name="accp", bufs=1) as acc_pool,
            tc.tile_pool(name="smallp", bufs=2) as small_pool,
            tc.tile_pool(name="outp", bufs=2) as out_pool,
            tc.tile_pool(name="denp", bufs=1, space="PSUM") as den_pool,
        ):
            for ch in range(CHUNKS):
                I = [img_pool.tile([128, EY, EX], bf16, tag=f"I{c}", name=f"I{c}") for c in range(C)]
                for c in range(C):
                    nc.sync.dma_start(I[c][:], inp[ch, c])

                acc = [acc_pool.tile([128, BR, BC], f32, tag=f"acc{c}", name=f"acc{c}") for c in range(C)]
                den = den_pool.tile([128, BR, BC], f32, tag="den", name="den")
                for c in range(C):
                    nc.vector.memset(acc[c][:], 0.0)
                nc.vector.memset(den[:], 1.0)  # center tap weight

                for dy, dx in PAIRS:
                    # Weight-field region R_t (tile coords): origin (ry,cx),
                    # size (sy,sx). Covers output pixels and output-minus-t.
                    ry = R - dy
                    cx = R - max(dx, 0)
                    sy = BR + dy
                    sx = BC + abs(dx)

                    dt_ = [d_pool.tile([128, EY, EX], bf16, tag=f"d{c}", name=f"d{c}") for c in range(C)]
                    ab = [w_pool.tile([128, EY, EX], bf16, tag=f"ab{c}", name=f"ab{c}") for c in range(C)]
                    a0 = ab[0]
                    s_a = w_pool.tile([128, EY, EX], bf16, tag="s_a", name="s_a")
                    s_b = w_pool.tile([128, EY, EX], bf16, tag="s_b", name="s_b")

                    for c in range(C):
                        # d_c = I_c(.+t) - I_c(.) on R_t, stored at origin 0.
                        nc.vector.tensor_sub(
                            dt_[c][:, :sy, :sx],
                            I[c][:, ry + dy : ry + dy + sy, cx + dx : cx + dx + sx],
                            I[c][:, ry : ry + sy, cx : cx + sx],
                        )
                    # s = |d0| + |d1| + |d2|  (abs on ACT, adds on DVE)
                    for c in range(C):
                        nc.scalar.activation(
                            ab[c][:, :sy, :sx], dt_[c][:, :sy, :sx], Act.Abs
                        )
                    nc.vector.tensor_add(
                        s_a[:, :sy, :sx], ab[0][:, :sy, :sx], ab[1][:, :sy, :sx]
                    )
                    nc.vector.tensor_add(
                        s_b[:, :sy, :sx], s_a[:, :sy, :sx], ab[2][:, :sy, :sx]
                    )
                    # W = exp(color_coeff * s^2 + ln(space_weight))  [ACT x2]
                    nc.scalar.activation(s_a[:, :sy, :sx], s_b[:, :sy, :sx], Act.Square)
                    nc.scalar.activation(
                        a0[:, :sy, :sx],
                        s_a[:, :sy, :sx],
                        Act.Exp,
                        bias=float(np.log(_space_weight(dy, dx))),
                        scale=COLOR_COEFF,
                    )
                    Wt = a0

                    # Gather/scatter views of an R_t-shaped field: gather
                    # (output pixels p) at origin (dy, cx0g); scatter (p-t)
                    # at origin (0, cx0s).
                    gy, gx = dy, max(dx, 0)
                    sy0, sx0 = 0, max(-dx, 0)

                    for c in range(C):
                        G = g_pool.tile([128, EY, EX], bf16, tag="G", name="G")
                        nc.vector.tensor_mul(
                            G[:, :sy, :sx], Wt[:, :sy, :sx], dt_[c][:, :sy, :sx]
                        )
                        t1 = small_pool.tile([128, BR, BC], bf16, tag="t1", name="t1")
                        nc.vector.tensor_sub(
                            t1[:],
                            G[:, gy : gy + BR, gx : gx + BC],
                            G[:, sy0 : sy0 + BR, sx0 : sx0 + BC],
                        )
                        nc.vector.tensor_add(acc[c][:], acc[c][:], t1[:])
                    wsum = small_pool.tile([128, BR, BC], bf16, tag="wsum", name="wsum")
                    nc.vector.tensor_add(
                        wsum[:],
                        Wt[:, gy : gy + BR, gx : gx + BC],
                        Wt[:, sy0 : sy0 + BR, sx0 : sx0 + BC],
                    )
                    nc.vector.tensor_add(den[:], den[:], wsum[:])

                # Epilogue: out_c = center_c + acc_c / den
                rec = small_pool.tile([128, BR, BC], f32, tag="rec", name="rec")
                nc.vector.reciprocal(rec[:], den[:])
                for c in range(C):
                    nc.vector.tensor_mul(acc[c][:], acc[c][:], rec[:])
                    ot = out_pool.tile([128, BR, BC], bf16, tag="ot", name="ot")
                    nc.vector.tensor_add(
                        ot[:], acc[c][:], I[c][:, R : R + BR, R : R + BC]
                    )
                    nc.sync.dma_start(outp[ch, c], ot[:])

    nc.compile()
    return nc


_NC_CACHE = {}


def _get_nc():
    if "nc" not in _NC_CACHE:
        _NC_CACHE["nc"] = _build_nc()
    return _NC_CACHE["nc"]


def _prep_core_input(img):
    """[H,W,C] f32 -> [CHUNKS, C, 128, EY, EX] bf16 halo windows."""
    padded = np.pad(img, ((R, R), (R, R), (0, 0)), mode="reflect")
    padded = np.ascontiguousarray(padded.transpose(2, 0, 1)).astype(BF16)  # [C,1032,1032]
    sw = np.lib.stride_tricks.sliding_window_view(padded, (EY, EX), axis=(1, 2))
    # sw[c, y0, x0] = padded[c, y0:y0+EY, x0:x0+EX]
    wins = sw[:, :: BR, :: BC]  # [C, 16, 32, EY, EX]
    wins = wins.reshape(C, ROW_BANDS, CHUNKS, COL_BLOCKS_PER_CHUNK, EY, EX)
    wins = wins.transpose(2, 0, 1, 3, 4, 5)  # [CHUNKS, C, 16, 8, EY, EX]
    return np.ascontiguousarray(wins).reshape(CHUNKS, C, 128, EY, EX)


def _assemble_core_output(out):
    """[CHUNKS, C, 128, BR, BC] bf16 -> [H,W,C] f32."""
    o = out.reshape(CHUNKS, C, ROW_BANDS, COL_BLOCKS_PER_CHUNK, BR, BC)
    o = o.transpose(1, 2, 4, 0, 3, 5)  # [C, 16, BR, CHUNKS, 8, BC]
    o = o.reshape(C, H, W).transpose(1, 2, 0)
    return np.ascontiguousarray(o, dtype=np.float32)


def kernel(sample):
    sample = np.asarray(sample, dtype=np.float32)
    assert sample.shape == (B, H, W, C)
    nc = _get_nc()
    in_maps = [{"win": _prep_core_input(sample[i])} for i in range(B)]
    res = run_bass_kernel_spmd(nc, in_maps, list(range(N_CORES)))
    return np.stack(
        [_assemble_core_output(res.results[i]["out"]) for i in range(B)], axis=0
    )


if __name__ == "__main__":
    x = np.random.RandomState(0).rand(B, H, W, C).astype(np.float32) * 255.0
    y = kernel(x)
    print("kernel output:", y.shape, y.dtype, float(y.min()), float(y.max()))
